# revision 1
# baseline (speedup 1.0000x reference)
import sys
sys.path.insert(0, "/opt/trn_rl_repo")
import numpy as np
import ml_dtypes
from contextlib import ExitStack

import concourse.bass as bass
import concourse.bacc as bacc
import concourse.tile as tile
from concourse import mybir
from concourse.bass_utils import run_bass_kernel_spmd

F32 = mybir.dt.float32
F32R = mybir.dt.float32r
BF16 = mybir.dt.bfloat16
AF = mybir.ActivationFunctionType

B, NQ, NK, DV, H, D = 4, 1024, 1024, 1024, 16, 64
QS = 512          # queries per core
EPS = 1e-5
SCALE = 1.0 / 32.0  # 1/sqrt(DV)

_CACHE = {}


def _build():
    nc = bacc.Bacc("TRN2", target_bir_lowering=False)

    qt_d = nc.dram_tensor("qt", [DV, QS], F32, kind="ExternalInput")
    kt_d = nc.dram_tensor("kt", [DV, NK], F32, kind="ExternalInput")
    wq_d = nc.dram_tensor("wq", [DV, DV], BF16, kind="ExternalInput")
    wk_d = nc.dram_tensor("wk", [DV, DV], BF16, kind="ExternalInput")
    wv_d = nc.dram_tensor("wv", [DV, DV], BF16, kind="ExternalInput")
    wo_d = nc.dram_tensor("wo", [DV, DV], BF16, kind="ExternalInput")
    vqc_d = nc.dram_tensor("vqc", [128, 8], F32, kind="ExternalInput")
    vkc_d = nc.dram_tensor("vkc", [128, 8], F32, kind="ExternalInput")
    voc_d = nc.dram_tensor("voc", [128, 8], F32, kind="ExternalInput")
    vvr_d = nc.dram_tensor("vvr", [1, DV], BF16, kind="ExternalInput")
    ones_d = nc.dram_tensor("onesd", [128, 128], F32, kind="ExternalInput")
    out_d = nc.dram_tensor("out", [DV, QS], F32, kind="ExternalOutput")

    with tile.TileContext(nc) as tc, ExitStack() as ctx:
        ctx.enter_context(nc.allow_low_precision(reason="bf16 softmax/stats by design"))
        P = ctx.enter_context  # shorthand
        pool = P(tc.tile_pool(name="main", bufs=1))
        pw = P(tc.tile_pool(name="w", bufs=8))
        pbig = P(tc.tile_pool(name="big", bufs=8))
        pkn = P(tc.tile_pool(name="kn", bufs=8))
        phalf = P(tc.tile_pool(name="half", bufs=8))
        pat = P(tc.tile_pool(name="at", bufs=8))
        ptmp = P(tc.tile_pool(name="tmp", bufs=2))
        psq = P(tc.tile_pool(name="sq", bufs=3))
        pst = P(tc.tile_pool(name="pst", bufs=2, space="PSUM"))
        pp1 = P(tc.tile_pool(name="pp1", bufs=2, space="PSUM"))
        pss = P(tc.tile_pool(name="pss", bufs=2, space="PSUM"))

        # ---- constants ----
        onesr = pool.tile([128, 128], F32R, tag="onesr")
        nc.sync.dma_start(onesr[:], ones_d[:].bitcast(F32R))
        ones16 = pool.tile([1, 128], BF16, tag="ones16")
        nc.vector.memset(ones16[:], 1.0)
        vqc = pool.tile([128, 8], F32, tag="vqc")
        nc.sync.dma_start(vqc[:], vqc_d[:])
        vkc = pool.tile([128, 8], F32, tag="vkc")
        nc.sync.dma_start(vkc[:], vkc_d[:])
        voc = pool.tile([128, 8], F32, tag="voc")
        nc.sync.dma_start(voc[:], voc_d[:])
        vvr = pool.tile([1, DV], BF16, tag="vvr")
        nc.sync.dma_start(vvr[:], vvr_d[:])
        mrow_q = pool.tile([1, QS], BF16, tag="mrow_q")
        irow_q = pool.tile([1, QS], BF16, tag="irow_q")
        mrow_k = pool.tile([1, NK], BF16, tag="mrow_k")
        irow_k = pool.tile([1, NK], BF16, tag="irow_k")
        mrow_o = pool.tile([1, QS], BF16, tag="mrow_o")
        irow_o = pool.tile([1, QS], BF16, tag="irow_o")
        rrow = [pool.tile([1, 512], BF16, tag=f"rrow{i}", name=f"rrow{i}")
                for i in range(4)]
        epst = pool.tile([1, 1], F32, tag="epst")
        nc.vector.memset(epst[:], EPS)
        m32r = pool.tile([1, NK], F32, tag="m32r")
        m2r = pool.tile([1, NK], F32, tag="m2r")
        varr = pool.tile([1, NK], F32, tag="varr")
        sr = pool.tile([1, NK], F32, tag="sr")

        # ---- load inputs ----
        qt = [pool.tile([128, QS], F32R, tag=f"qt{f}", name=f"qt{f}") for f in range(8)]
        for f in range(8):
            nc.sync.dma_start(qt[f][:], qt_d[f * 128:(f + 1) * 128, :].bitcast(F32R))
        kt = [pbig.tile([128, NK], F32R, tag="kt", name=f"kt{f}") for f in range(8)]
        for f in range(8):
            nc.sync.dma_start(kt[f][:], kt_d[f * 128:(f + 1) * 128, :].bitcast(F32R))

        def stats(xtiles, n_tok, nchunks, mrow, irow, scale_n):
            # per-token mean/invstd rows (bf16) from feature-major tiles
            for c in range(nchunks):
                s0 = pst.tile([1, 512], F32, tag="st")
                s1 = pst.tile([1, 512], F32, tag="st")
                for f in range(8):
                    xs = xtiles[f][:, c * 512:(c + 1) * 512]
                    sq = psq.tile([128, 512], BF16, tag="sq")
                    nc.gpsimd.tensor_mul(sq[:], xs, xs)
                    nc.tensor.matmul(s0[:], onesr[:, 0:1], xs,
                                     start=(f == 0), stop=(f == 7))
                    nc.tensor.matmul(s1[:], ones16_col[:, 0:1], sq[:],
                                     start=(f == 0), stop=(f == 7))
                cs = slice(c * 512, (c + 1) * 512)
                m32 = m32r[0:1, cs]
                nc.scalar.activation(m32, s0[:], AF.Copy, scale=scale_n)
                nc.scalar.activation(mrow[0:1, cs], s0[:], AF.Copy, scale=scale_n)
                m2 = m2r[0:1, cs]
                nc.vector.tensor_mul(m2, m32, m32)
                var = varr[0:1, cs]
                nc.vector.scalar_tensor_tensor(var, s1[:], scale_n, m2,
                                               op0=mybir.AluOpType.mult,
                                               op1=mybir.AluOpType.subtract)
                srow = sr[0:1, cs]
                nc.scalar.activation(srow, var, AF.Sqrt, bias=epst[:])
                nc.vector.reciprocal(irow[0:1, cs], srow)

        ones16_col = pool.tile([128, 1], BF16, tag="o16c")
        nc.vector.memset(ones16_col[:], 1.0)

        stats(qt, QS, 1, mrow_q, irow_q, 1.0 / DV)
        stats(kt, NK, 2, mrow_k, irow_k, 1.0 / DV)

        def prenorm(xtiles, out_tiles, nchunks, mrow, irow):
            for c in range(nchunks):
                cs = slice(c * 512, (c + 1) * 512)
                bm = pp1.tile([128, 512], F32, tag="bc")
                nc.tensor.matmul(bm[:], ones16[0:1, :], mrow[0:1, cs],
                                 start=True, stop=True)
                bi = pp1.tile([128, 512], F32, tag="bc")
                nc.tensor.matmul(bi[:], ones16[0:1, :], irow[0:1, cs],
                                 start=True, stop=True)
                for f in range(8):
                    o = out_tiles[f][:, cs]
                    nc.vector.tensor_sub(o, xtiles[f][:, cs], bm[:])
                    nc.vector.tensor_mul(o, o, bi[:])

        qn = [phalf.tile([128, QS], BF16, tag="hn", name=f"qn{i}") for i in range(8)]
        prenorm(qt, qn, 1, mrow_q, irow_q)
        kn = [pkn.tile([128, NK], BF16, tag="kn", name=f"kn{i}") for i in range(8)]
        prenorm(kt, kn, 2, mrow_k, irow_k)

        # ---- projections ----
        wq = [pw.tile([128, DV], BF16, tag="w", name=f"wq{i}") for i in range(8)]
        for f in range(8):
            nc.sync.dma_start(wq[f][:], wq_d[f * 128:(f + 1) * 128, :])
        qp = [pool.tile([128, QS], BF16, tag=f"qp{m}", name=f"qp{m}") for m in range(8)]
        for m in range(8):
            ps = pp1.tile([128, 512], F32, tag="pp")
            for f in range(8):
                nc.tensor.matmul(ps[:], wq[f][:, m * 128:(m + 1) * 128], qn[f][:],
                                 start=(f == 0), stop=(f == 7))
            nc.scalar.activation(qp[m][:], ps[:], AF.Identity, bias=vqc[:, m:m + 1])

        wk = [pw.tile([128, DV], BF16, tag="w", name=f"wk{i}") for i in range(8)]
        for f in range(8):
            nc.sync.dma_start(wk[f][:], wk_d[f * 128:(f + 1) * 128, :])
        kp = [pool.tile([128, NK], BF16, tag=f"kp{m}", name=f"kp{m}") for m in range(8)]
        for m in range(8):
            for c in range(2):
                ps = pp1.tile([128, 512], F32, tag="pp")
                for f in range(8):
                    nc.tensor.matmul(ps[:], wk[f][:, m * 128:(m + 1) * 128],
                                     kn[f][:, c * 512:(c + 1) * 512],
                                     start=(f == 0), stop=(f == 7))
                nc.scalar.activation(kp[m][:, c * 512:(c + 1) * 512], ps[:],
                                     AF.Identity, bias=vkc[:, m:m + 1])

        wv = [pw.tile([128, DV], BF16, tag="w", name=f"wv{i}") for i in range(8)]
        for f in range(8):
            nc.sync.dma_start(wv[f][:], wv_d[f * 128:(f + 1) * 128, :])
        # Vp token-major with 65-stride head slots (64 data + 1 ones col)
        vp = [pool.tile([128, 1040], BF16, tag=f"vp{t}", name=f"vp{t}") for t in range(8)]
        for t in range(8):
            nc.vector.memset(
                vp[t][:].rearrange("p (s e) -> p s e", e=65)[:, :, 64:65], 1.0)
            for c in range(2):
                ps = pp1.tile([128, 512], F32, tag="pp")
                for f in range(8):
                    nc.tensor.matmul(ps[:], kn[f][:, t * 128:(t + 1) * 128],
                                     wv[f][:, c * 512:(c + 1) * 512],
                                     start=(f == 0), stop=False)
                nc.tensor.matmul(ps[:], ones16[0:1, :],
                                 vvr[:, c * 512:(c + 1) * 512],
                                 start=False, stop=True)
                dst = vp[t][:, c * 520:c * 520 + 520].rearrange(
                    "p (s e) -> p s e", e=65)[:, :, 0:64]
                nc.vector.tensor_copy(dst, ps[:].rearrange("p (s e) -> p s e", e=64))

        # ---- attention + output assembly ----
        wo = [pw.tile([128, DV], BF16, tag="w", name=f"wo{i}") for i in range(8)]
        for f in range(8):
            nc.sync.dma_start(wo[f][:], wo_d[f * 128:(f + 1) * 128, :])

        o = [pool.tile([128, QS], F32R, tag=f"o{t}", name=f"o{t}") for t in range(8)]
        for h in range(H):
            dt_, po = h // 2, (h % 2) * 64
            at = []
            for j in range(4):
                a = pat.tile([128, 1024], BF16, tag="at")
                for half in range(2):
                    k8 = 2 * j + half
                    ss = pss.tile([128, 512], F32, tag="ss")
                    nc.tensor.matmul(
                        ss[:],
                        kp[dt_][po:po + 64, k8 * 128:(k8 + 1) * 128],
                        qp[dt_][po:po + 64, :], start=True, stop=True)
                    nc.scalar.activation(a[:, half * 512:(half + 1) * 512],
                                         ss[:], AF.Exp, scale=SCALE)
                at.append(a)
            cc = pst.tile([128, 512], F32, tag="st")
            sbase = (h // 8) * 520 + (h % 8) * 65
            for k8 in range(8):
                nc.tensor.matmul(cc[0:65, :], vp[k8][:, sbase:sbase + 65],
                                 at[k8 // 2][:, (k8 % 2) * 512:(k8 % 2) * 512 + 512],
                                 start=(k8 == 0), stop=(k8 == 7))
            rr = rrow[h % 4][:]
            nc.vector.reciprocal(rr, cc[64:65, :])
            pb = pp1.tile([64, 512], F32, tag="bc")
            nc.tensor.matmul(pb[:], ones16[0:1, 0:64], rr, start=True, stop=True)
            bs = ptmp.tile([64, 512], F32, tag="bs")
            nc.vector.tensor_copy(bs[:], pb[:])
            nc.vector.tensor_mul(o[dt_][po:po + 64, :], cc[0:64, :], bs[:])

        for t in range(8):
            nc.vector.tensor_add(o[t][:], o[t][:], qt[t][:])

        # ---- LN(O) + output proj ----
        stats(o, QS, 1, mrow_o, irow_o, 1.0 / DV)
        on = [phalf.tile([128, QS], BF16, tag="hn", name=f"on{i}") for i in range(8)]
        prenorm(o, on, 1, mrow_o, irow_o)
        for m in range(8):
            ps = pp1.tile([128, 512], F32, tag="pp")
            for f in range(8):
                nc.tensor.matmul(ps[:], wo[f][:, m * 128:(m + 1) * 128], on[f][:],
                                 start=(f == 0), stop=(f == 7))
            ro = ptmp.tile([128, 512], F32, tag="ro")
            nc.scalar.activation(ro[:], ps[:], AF.Relu, bias=voc[:, m:m + 1])
            ob = ptmp.tile([128, 512], F32, tag="ob")
            nc.gpsimd.tensor_add(ob[:], ro[:], o[m][:])
            nc.sync.dma_start(out_d[m * 128:(m + 1) * 128, :], ob[:])

    nc.compile()
    return nc


def kernel(**inputs):
    Q = np.asarray(inputs["Q"], np.float32)
    K = np.asarray(inputs["K"], np.float32)
    wq, bq = np.asarray(inputs["wq"], np.float32), np.asarray(inputs["bq"], np.float32)
    wk, bk = np.asarray(inputs["wk"], np.float32), np.asarray(inputs["bk"], np.float32)
    wv, bv = np.asarray(inputs["wv"], np.float32), np.asarray(inputs["bv"], np.float32)
    wo, bo = np.asarray(inputs["wo"], np.float32), np.asarray(inputs["bo"], np.float32)
    gq, betaq = np.asarray(inputs["gq"], np.float32), np.asarray(inputs["betaq"], np.float32)
    gk, betak = np.asarray(inputs["gk"], np.float32), np.asarray(inputs["betak"], np.float32)
    g0, beta0 = np.asarray(inputs["g0"], np.float32), np.asarray(inputs["beta0"], np.float32)

    if "nc" not in _CACHE:
        _CACHE["nc"] = _build()
    nc = _CACHE["nc"]

    BF = ml_dtypes.bfloat16
    wq_e = np.ascontiguousarray((gq[:, None] * wq).astype(BF))
    wk_e = np.ascontiguousarray((gk[:, None] * wk).astype(BF))
    wv_e = np.ascontiguousarray((gk[:, None] * wv).astype(BF))
    wo_e = np.ascontiguousarray((g0[:, None] * wo).astype(BF))
    vq = (betaq @ wq + bq).reshape(8, 128).T.copy()
    vk = (betak @ wk + bk).reshape(8, 128).T.copy()
    vo = (beta0 @ wo + bo).reshape(8, 128).T.copy()
    vv = (betak @ wv + bv).reshape(1, DV).astype(ml_dtypes.bfloat16)
    ones = np.ones((128, 128), np.float32)

    shared = {"wq": wq_e, "wk": wk_e, "wv": wv_e, "wo": wo_e,
              "vqc": vq, "vkc": vk, "voc": vo, "vvr": vv, "onesd": ones}
    in_maps = []
    for c in range(8):
        b, q0 = c // 2, (c % 2) * QS
        m = dict(shared)
        m["qt"] = np.ascontiguousarray(Q[b, q0:q0 + QS, :].T)
        m["kt"] = np.ascontiguousarray(K[b].T)
        in_maps.append(m)

    _CACHE["in_map0"] = in_maps[0]
    trace = _CACHE.get("trace", False)
    res = run_bass_kernel_spmd(nc, in_maps, list(range(8)), trace=trace)
    _CACHE["last"] = res

    out = np.empty((B, NQ, DV), np.float32)
    for c in range(8):
        b, q0 = c // 2, (c % 2) * QS
        out[b, q0:q0 + QS, :] = res.results[c]["out"].T
    return out

